# revision 9
# baseline (speedup 1.0000x reference)
"""Causal multi-head self-attention with RoPE on 8 TRN2 NeuronCores.

Problem (hardcoded): B=2, S=2048, D=1024, H=16, d_k=64, fp32 I/O.

Sharding (data + tensor parallel): core c -> batch c//4, head group c%4
(4 heads = 256 dims). Wq/Wk/Wv column-split, Wo row-split; host sums the
4 partial [S, D] outputs per batch (bf16 partials, fp32 accumulation).

v3 structure:
  - All 4 heads projected chunk-by-chunk (512 tokens); attention column j
    streams right after chunk j.
  - Attention inner loop software-pipelined: score pair for sk-tile i is
    emitted together with the P@V pair for tile i-1, so the PE stream is
    [S S PV PV] per group and exp (ACT) overlaps the PE work.
  - Projection / V / output-projection pieces are interleaved between
    attention groups as PE filler (keeps PE warm through exp stalls and
    column transitions).
  - Score matmuls (contraction 64) issued as adjacent pairs on PE row
    groups 0/64 so they co-execute on the tiled PE array.
  - RoPE rotate-half partner lives 16 rows away inside each 32-partition
    quadrant (host-side weight row permutation): partner tensor comes from
    a DVE stream_shuffle; cos/sin multiplies are fused into the PSUM
    eviction (DVE tensor ops reading PSUM); final add on GpSimd.
  - Softmax denominator rides as the 65th output row of P@V (ones column
    in V); triangular causal mask applied post-exp on GpSimd.
  - Output written bf16 (halves the output DMA); host sums in fp32.
"""

import numpy as np
import ml_dtypes

B, S, D = 2, 2048, 1024
H, DK = 16, 64
HPC = 4          # heads per core
E = HPC * DK     # 256 output dims per core
P = 128
KS = D // P      # 8 contraction subtiles
SC = 512         # s-chunk width
NSC = S // SC    # 4 chunks (also the 4 attention columns)
NSK = S // P     # 16 sk tiles
BF = ml_dtypes.bfloat16
SHUF16 = list(range(16, 32)) + list(range(0, 16))

_CACHE = {}


def _build_nc():
    import concourse.bacc as bacc
    import concourse.mybir as mybir
    import concourse.tile as tile
    from contextlib import ExitStack

    bf = mybir.dt.bfloat16
    f32 = mybir.dt.float32
    Exp = mybir.ActivationFunctionType.Exp

    nc = bacc.Bacc("TRN2", target_bir_lowering=False)

    xT = nc.dram_tensor("xT", [D, S], bf, kind="ExternalInput")
    wq = nc.dram_tensor("wq", [D, E], bf, kind="ExternalInput")
    wk = nc.dram_tensor("wk", [D, E], bf, kind="ExternalInput")
    wv = nc.dram_tensor("wv", [D, E], bf, kind="ExternalInput")
    wo = nc.dram_tensor("wo", [E, D], bf, kind="ExternalInput")
    cs = nc.dram_tensor("cs", [P, S], bf, kind="ExternalInput")
    sn = nc.dram_tensor("sn", [P, S], bf, kind="ExternalInput")
    tri = nc.dram_tensor("tri", [P, P], bf, kind="ExternalInput")
    out = nc.dram_tensor("out", [S, D], bf, kind="ExternalOutput")

    from concourse import library_config

    with tile.TileContext(nc) as tc, ExitStack() as ctx:
        # one gpsimd ucode library for tensor_tensor AND partition_broadcast:
        # avoids mid-kernel library reload thrash on the Pool engine
        try:
            nc.gpsimd.load_library(library_config.proxy)
        except Exception:
            pass
        const = ctx.enter_context(tc.tile_pool(name="const", bufs=1))
        work = ctx.enter_context(tc.tile_pool(name="work", bufs=3))
        pexp_pool = ctx.enter_context(tc.tile_pool(name="pexpp", bufs=6))
        pa = ctx.enter_context(tc.tile_pool(name="pa", bufs=2, space="PSUM"))
        pb = ctx.enter_context(tc.tile_pool(name="pb", bufs=2, space="PSUM"))
        pvp = ctx.enter_context(tc.tile_pool(name="pvp", bufs=1, space="PSUM"))

        # ---- input DMAs: wq + chunk 0 first so compute starts early ----
        xTv = xT.rearrange("(ks p) s -> p ks s", p=P)
        wq_sb = const.tile([P, KS, E], bf, tag="wq")
        nc.sync.dma_start(wq_sb[:], wq.rearrange("(ks p) e -> p ks e", p=P))
        xss = []
        for st in range(NSC):
            xc = const.tile([P, KS, SC], bf, tag=f"xs{st}", name=f"xs{st}")
            xss.append(xc)
        nc.sync.dma_start(xss[0][:], xTv[:, :, 0:SC])
        cs_sb = const.tile([P, S], bf, tag="cs")
        nc.sync.dma_start(cs_sb[:], cs[:])
        sn_sb = const.tile([P, S], bf, tag="sn")
        nc.sync.dma_start(sn_sb[:], sn[:])
        wk_sb = const.tile([P, KS, E], bf, tag="wk")
        nc.sync.dma_start(wk_sb[:], wk.rearrange("(ks p) e -> p ks e", p=P))
        wv_sb = const.tile([P, KS, E], bf, tag="wv")
        nc.sync.dma_start(wv_sb[:], wv.rearrange("(ks p) e -> p ks e", p=P))
        tri_sb = const.tile([P, P], bf, tag="tri")
        nc.sync.dma_start(tri_sb[:], tri[:])
        nc.sync.dma_start(xss[1][:], xTv[:, :, SC:2 * SC])
        wo_sb = const.tile([P, 2, D], bf, tag="wo")
        nc.sync.dma_start(wo_sb[:], wo.rearrange("(ks p) e -> p ks e", p=P))
        nc.sync.dma_start(xss[2][:], xTv[:, :, 2 * SC:3 * SC])
        nc.sync.dma_start(xss[3][:], xTv[:, :, 3 * SC:4 * SC])

        # persistent tensors
        qts = [const.tile([P, S], bf, tag=f"qt{eb}", name=f"qt{eb}") for eb in range(2)]
        kts = [const.tile([P, S], bf, tag=f"kt{eb}", name=f"kt{eb}") for eb in range(2)]
        # V augmented with a ones column per head: [s-part, sk-tile, 4*(64+1)]
        vaug = const.tile([P, NSK, HPC * (DK + 1)], bf, tag="vaug")
        vaug4 = vaug.rearrange("p t (h e) -> p t h e", h=HPC)
        nc.vector.memset(vaug4[:, :, :, DK], 1.0)
        # normalized attention values, laid out as Wo lhsT [e%128, e//128, s]
        vals = const.tile([P, 2, S], bf, tag="vals")

        def proj_qk_piece(w_sb, eb, sc, dst):
            """dst[:, chunk sc] = rope(W.T @ x.T) for e-block eb."""
            cols = slice(sc * SC, (sc + 1) * SC)
            ps = pa.tile([P, SC], f32, tag="pa")
            for ks in range(KS):
                nc.tensor.matmul(
                    ps[:],
                    lhsT=w_sb[:, ks, eb * P:(eb + 1) * P],
                    rhs=xss[sc][:, ks, :],
                    start=(ks == 0), stop=(ks == KS - 1),
                )
            t = work.tile([P, SC], bf, tag="t")
            nc.vector.tensor_mul(out=t[:], in0=ps[:], in1=cs_sb[:, cols])
            v = work.tile([P, SC], bf, tag="v")
            nc.vector.tensor_mul(out=v[:], in0=ps[:], in1=sn_sb[:, cols])
            u = work.tile([P, SC], bf, tag="u")
            nc.vector.stream_shuffle(out=u[:], in_=v[:], mask=SHUF16)
            nc.gpsimd.tensor_add(out=dst[:, cols], in0=t[:], in1=u[:])

        def proj_v_piece(sc, s4):
            """V for one 128-token block into its vaug slot."""
            sst = sc * 4 + s4
            ps = pa.tile([P, SC], f32, tag="pa")
            pv256 = ps[:, 0:E]
            for ks in range(KS):
                nc.tensor.matmul(
                    pv256,
                    lhsT=xss[sc][:, ks, s4 * P:(s4 + 1) * P],
                    rhs=wv_sb[:, ks, :],
                    start=(ks == 0), stop=(ks == KS - 1),
                )
            nc.vector.tensor_copy(
                out=vaug4[:, sst, :, 0:DK],
                in_=pv256.rearrange("p (h e) -> p h e", h=HPC),
            )

        def outproj_piece(j, s4, tail):
            """out[sq block, :] = vals.T @ woT for 128 tokens."""
            sq = j * 4 + s4
            for n2 in range(2):
                ps = pa.tile([P, SC], f32, tag="pa")
                for ks2 in range(2):
                    nc.tensor.matmul(
                        ps[:],
                        lhsT=vals[:, ks2, sq * P:(sq + 1) * P],
                        rhs=wo_sb[:, ks2, n2 * SC:(n2 + 1) * SC],
                        start=(ks2 == 0), stop=(ks2 == 1),
                    )
                ostg = work.tile([P, SC], bf, tag="ostg", name="ostg")
                if tail and n2 == 1:
                    nc.scalar.copy(out=ostg[:], in_=ps[:])
                else:
                    nc.vector.tensor_copy(out=ostg[:], in_=ps[:])
                nc.sync.dma_start(
                    out=out[sq * P:(sq + 1) * P, n2 * SC:(n2 + 1) * SC],
                    in_=ostg[:])

        def attention_col(j, hp, filler, fill_every):
            """Attention for sq column j, head pair hp (heads 2hp, 2hp+1).

            Emits score pair i together with P@V pair i-1 (one-group software
            pipeline); pulls a filler piece every `fill_every` groups.
            """
            jcols = slice(j * SC, (j + 1) * SC)
            ntiles = 4 * j + 4
            pvts = [pvp.tile([P, SC], f32, tag=f"pv{par}", name=f"pv{par}")
                    for par in range(2)]

            def emit_pv(prev):
                pexp, off, i = prev
                for par in range(2):
                    hl = 2 * hp + par
                    nc.tensor.matmul(
                        pvts[par][0:DK + 1, off:],
                        lhsT=vaug[:, i, hl * (DK + 1):(hl + 1) * (DK + 1)],
                        rhs=pexp[:, par, off:],
                        start=(i == 0), stop=(i == ntiles - 1),
                    )

            pend = []
            for i in range(ntiles):
                c = i - 4 * j
                off = c * P if c > 0 else 0
                stp = pb.tile([P, 2, SC], f32, tag="pb", name="stp")
                for par in range(2):
                    nc.tensor.matmul(
                        stp[:, par, off:],
                        lhsT=kts[hp][64 * par:64 * par + 64, i * P:(i + 1) * P],
                        rhs=qts[hp][64 * par:64 * par + 64,
                                    j * SC + off:(j + 1) * SC],
                        start=True, stop=True,
                    )
                if len(pend) >= 2:
                    emit_pv(pend.pop(0))
                pexp = pexp_pool.tile([P, 2, SC], bf, tag="pexp", name="pexp")
                nc.scalar.activation(out=pexp[:, :, off:], in_=stp[:, :, off:],
                                     func=Exp)
                if c >= 0:  # diagonal subtile: triangular mask
                    for par in range(2):
                        sl = pexp[:, par, c * P:(c + 1) * P]
                        nc.gpsimd.tensor_mul(out=sl, in0=sl, in1=tri_sb[:])
                pend.append((pexp, off, i))
                if filler and (i % fill_every == fill_every - 1):
                    filler.pop(0)()
            while pend:
                emit_pv(pend.pop(0))

            # normalize by the softmax denominator (row DK of pvts)
            for par in range(2):
                rsb = work.tile([P, SC], f32, tag="rsb")
                rb = work.tile([P, SC], f32, tag="rb")
                lsb = work.tile([P, SC], f32, tag="lsb", name="lsb")
                # cross-partition DVE copy: PSUM p64 -> SBUF p0; fast
                # reciprocal works from SBUF p0. Keeps ACT free for exp.
                nc.vector.tensor_copy(out=lsb[0:1, :], in_=pvts[par][DK:DK + 1, :])
                nc.vector.reciprocal_approx_fast(out=rsb[0:1, :],
                                                 in_=lsb[0:1, :])
                nc.gpsimd.partition_broadcast(rb[0:DK, :], rsb[0:1, :],
                                              channels=DK)
                dst = vals[64 * par:64 * par + 64, hp, jcols]
                if par == 0:
                    nc.vector.tensor_mul(out=dst, in0=pvts[par][0:DK, :],
                                         in1=rb[0:DK, :])
                else:
                    stg = work.tile([P, SC], bf, tag="stg")
                    nc.vector.tensor_mul(out=stg[0:DK, :], in0=pvts[par][0:DK, :],
                                         in1=rb[0:DK, :])
                    nc.sync.dma_start(out=dst, in_=stg[0:DK, :])

        # chunk 0 Q/K projections up front; V(0) becomes column-0 filler
        proj_qk_piece(wq_sb, 0, 0, qts[0])
        proj_qk_piece(wk_sb, 0, 0, kts[0])
        proj_qk_piece(wq_sb, 1, 0, qts[1])
        proj_qk_piece(wk_sb, 1, 0, kts[1])

        opt = []  # deferrable outproj pieces (vals persist; roll backwards)
        for sc in range(NSC):
            mand = []  # must complete during this sc iteration
            if sc == 0:
                for s4 in range(4):
                    mand.append(lambda s4=s4: proj_v_piece(0, s4))
            if sc + 1 < NSC:
                mand.append(lambda sc=sc: proj_qk_piece(wq_sb, 0, sc + 1, qts[0]))
                mand.append(lambda sc=sc: proj_qk_piece(wk_sb, 0, sc + 1, kts[0]))
                mand.append(lambda sc=sc: proj_qk_piece(wq_sb, 1, sc + 1, qts[1]))
                mand.append(lambda sc=sc: proj_qk_piece(wk_sb, 1, sc + 1, kts[1]))
                for s4 in range(4):
                    mand.append(lambda sc=sc, s4=s4: proj_v_piece(sc + 1, s4))
            ng = 4 * sc + 4
            h = (len(mand) + 1) // 2
            f0 = mand[:h] + opt[:1]
            f1 = mand[h:] + opt[1:2]
            del opt[:2]
            fe0 = max(1, ng // (len(f0) + 1)) if f0 else ng
            fe1 = max(1, ng // (len(f1) + 1)) if f1 else ng
            attention_col(sc, 0, f0, fe0)
            attention_col(sc, 1, f1, fe1)
            for f in f0 + f1:  # leftovers must land before next chunk's column
                f()
            if sc >= 1:
                for s4 in range(4):
                    opt.append(lambda sc=sc, s4=s4: outproj_piece(sc - 1, s4, False))
            if opt and sc + 1 < NSC:
                opt.pop(0)()
        for f in opt:
            f()
        for s4 in range(4):
            outproj_piece(NSC - 1, s4, True)

    nc.compile()
    return nc


def get_nc():
    if "nc" not in _CACHE:
        _CACHE["nc"] = _build_nc()
    return _CACHE["nc"]


def make_in_maps(x, Wq, Wk, Wv, Wo, token_positions, rope_theta):
    """Host-side sharding: per-core input dict (bf16, pre-transposed/permuted)."""
    x = np.asarray(x, np.float32)
    Wq = np.asarray(Wq, np.float32)
    Wk = np.asarray(Wk, np.float32)
    Wv = np.asarray(Wv, np.float32)
    Wo = np.asarray(Wo, np.float32)
    pos = np.asarray(token_positions).astype(np.float32)
    theta = float(np.asarray(rope_theta))

    # RoPE row layout per head (64 rows = 2 SBUF quadrants of 32):
    # quadrant q holds [evens of pairs 16q..16q+15, odds of same pairs], so
    # the rotate-half partner is 16 rows away inside the same quadrant.
    perm = []
    sign = np.empty(DK, np.float32)
    pairidx = np.empty(DK, np.int64)
    r = 0
    for q in range(2):
        for p in range(16 * q, 16 * q + 16):
            perm.append(2 * p)
            sign[r] = -1.0
            pairidx[r] = p
            r += 1
        for p in range(16 * q, 16 * q + 16):
            perm.append(2 * p + 1)
            sign[r] = 1.0
            pairidx[r] = p
            r += 1
    perm = np.array(perm)

    freqs = theta ** (-np.arange(DK // 2, dtype=np.float32) / (DK // 2))
    ang = pos[:, None] * freqs[None, :]          # [S, 32]
    cos_t = np.cos(ang).T.astype(np.float32)     # [32, S]
    sin_t = np.sin(ang).T.astype(np.float32)
    cs64 = cos_t[pairidx]                        # [64, S]
    sn64 = sin_t[pairidx] * sign[:, None]        # [64, S]
    # sn multiplies the pre-shuffle tensor: shuffle(q0*sn_pre) == swap(q0)*sn
    sig = np.arange(DK)
    sig = (sig // 32) * 32 + ((sig % 32) + 16) % 32
    snp64 = sn64[sig]
    cs_t = np.tile(cs64, (2, 1)).astype(BF)      # [128, S]
    sn_t = np.tile(snp64, (2, 1)).astype(BF)

    tri_t = np.tril(np.ones((P, P), np.float32)).T.astype(BF)  # keep p<=f

    in_maps = []
    for c in range(8):
        b, g = c // 4, c % 4
        hs = slice(g * E, (g + 1) * E)

        def prep_qk(W, scale):
            Wl = W[hs].reshape(HPC, DK, D)[:, perm, :].reshape(E, D) * scale
            return np.ascontiguousarray(Wl.T).astype(BF)

        in_maps.append({
            "xT": np.ascontiguousarray(x[b].T).astype(BF),
            "wq": prep_qk(Wq, 1.0 / np.sqrt(DK)),
            "wk": prep_qk(Wk, 1.0),
            "wv": np.ascontiguousarray(Wv[hs].T).astype(BF),
            "wo": np.ascontiguousarray(Wo[:, hs].T).astype(BF),
            "cs": cs_t, "sn": sn_t, "tri": tri_t,
        })
    return in_maps


def kernel(x, Wq, Wk, Wv, Wo, token_positions, rope_theta):
    nc = get_nc()
    in_maps = make_in_maps(x, Wq, Wk, Wv, Wo, token_positions, rope_theta)
    from concourse.bass_utils import run_bass_kernel_spmd
    r = run_bass_kernel_spmd(nc, in_maps, core_ids=list(range(8)))
    outs = [np.asarray(m["out"], np.float32) for m in r.results]
    full = np.stack([sum(outs[0:4]), sum(outs[4:8])], 0)
    return full.astype(np.float32)


# revision 10
# speedup vs baseline: 1.0432x; 1.0432x over previous
"""Causal multi-head self-attention with RoPE on 8 TRN2 NeuronCores.

Problem (hardcoded): B=2, S=2048, D=1024, H=16, d_k=64, fp32 I/O.

Sharding (data + tensor parallel): core c -> batch c//4, head group c%4
(4 heads = 256 dims). Wq/Wk/Wv column-split, Wo row-split; host sums the
4 partial [S, D] outputs per batch (bf16 partials, fp32 accumulation).

v3 structure:
  - All 4 heads projected chunk-by-chunk (512 tokens); attention column j
    streams right after chunk j.
  - Attention inner loop software-pipelined: score pair for sk-tile i is
    emitted together with the P@V pair for tile i-1, so the PE stream is
    [S S PV PV] per group and exp (ACT) overlaps the PE work.
  - Projection / V / output-projection pieces are interleaved between
    attention groups as PE filler (keeps PE warm through exp stalls and
    column transitions).
  - Score matmuls (contraction 64) issued as adjacent pairs on PE row
    groups 0/64 so they co-execute on the tiled PE array.
  - RoPE rotate-half partner lives 16 rows away inside each 32-partition
    quadrant (host-side weight row permutation): partner tensor comes from
    a DVE stream_shuffle; cos/sin multiplies are fused into the PSUM
    eviction (DVE tensor ops reading PSUM); final add on GpSimd.
  - Softmax denominator rides as the 65th output row of P@V (ones column
    in V); triangular causal mask applied post-exp on GpSimd.
  - Output written bf16 (halves the output DMA); host sums in fp32.
"""

import numpy as np
import ml_dtypes

B, S, D = 2, 2048, 1024
H, DK = 16, 64
HPC = 4          # heads per core
E = HPC * DK     # 256 output dims per core
P = 128
KS = D // P      # 8 contraction subtiles
SC = 512         # s-chunk width
NSC = S // SC    # 4 chunks (also the 4 attention columns)
NSK = S // P     # 16 sk tiles
BF = ml_dtypes.bfloat16
SHUF16 = list(range(16, 32)) + list(range(0, 16))

_CACHE = {}


def _build_nc():
    import concourse.bacc as bacc
    import concourse.mybir as mybir
    import concourse.tile as tile
    from contextlib import ExitStack

    bf = mybir.dt.bfloat16
    f32 = mybir.dt.float32
    Exp = mybir.ActivationFunctionType.Exp

    nc = bacc.Bacc("TRN2", target_bir_lowering=False)

    xT = nc.dram_tensor("xT", [D, S], bf, kind="ExternalInput")
    wq = nc.dram_tensor("wq", [D, E], bf, kind="ExternalInput")
    wk = nc.dram_tensor("wk", [D, E], bf, kind="ExternalInput")
    wv = nc.dram_tensor("wv", [D, E], bf, kind="ExternalInput")
    wo = nc.dram_tensor("wo", [E, D], bf, kind="ExternalInput")
    cs = nc.dram_tensor("cs", [P, S], bf, kind="ExternalInput")
    sn = nc.dram_tensor("sn", [P, S], bf, kind="ExternalInput")
    tri = nc.dram_tensor("tri", [P, P], bf, kind="ExternalInput")
    out = nc.dram_tensor("out", [S, D], bf, kind="ExternalOutput")

    from concourse import library_config

    with tile.TileContext(nc) as tc, ExitStack() as ctx:
        # one gpsimd ucode library for tensor_tensor AND partition_broadcast:
        # avoids mid-kernel library reload thrash on the Pool engine
        try:
            nc.gpsimd.load_library(library_config.proxy)
        except Exception:
            pass
        const = ctx.enter_context(tc.tile_pool(name="const", bufs=1))
        work = ctx.enter_context(tc.tile_pool(name="work", bufs=3))
        pexp_pool = ctx.enter_context(tc.tile_pool(name="pexpp", bufs=6))
        pa = ctx.enter_context(tc.tile_pool(name="pa", bufs=2, space="PSUM"))
        pb = ctx.enter_context(tc.tile_pool(name="pb", bufs=2, space="PSUM"))
        pvp = ctx.enter_context(tc.tile_pool(name="pvp", bufs=1, space="PSUM"))

        # ---- input DMAs: wq + chunk 0 first so compute starts early ----
        xTv = xT.rearrange("(ks p) s -> p ks s", p=P)
        wq_sb = const.tile([P, KS, E], bf, tag="wq")
        nc.sync.dma_start(wq_sb[:], wq.rearrange("(ks p) e -> p ks e", p=P))
        xss = []
        for st in range(NSC):
            xc = const.tile([P, KS, SC], bf, tag=f"xs{st}", name=f"xs{st}")
            xss.append(xc)
        nc.sync.dma_start(xss[0][:], xTv[:, :, 0:SC])
        cs_sb = const.tile([P, S], bf, tag="cs")
        nc.sync.dma_start(cs_sb[:], cs[:])
        sn_sb = const.tile([P, S], bf, tag="sn")
        nc.sync.dma_start(sn_sb[:], sn[:])
        wk_sb = const.tile([P, KS, E], bf, tag="wk")
        nc.sync.dma_start(wk_sb[:], wk.rearrange("(ks p) e -> p ks e", p=P))
        wv_sb = const.tile([P, KS, E], bf, tag="wv")
        nc.sync.dma_start(wv_sb[:], wv.rearrange("(ks p) e -> p ks e", p=P))
        tri_sb = const.tile([P, P], bf, tag="tri")
        nc.sync.dma_start(tri_sb[:], tri[:])
        nc.sync.dma_start(xss[1][:], xTv[:, :, SC:2 * SC])
        wo_sb = const.tile([P, 2, D], bf, tag="wo")
        nc.sync.dma_start(wo_sb[:], wo.rearrange("(ks p) e -> p ks e", p=P))
        nc.sync.dma_start(xss[2][:], xTv[:, :, 2 * SC:3 * SC])
        nc.sync.dma_start(xss[3][:], xTv[:, :, 3 * SC:4 * SC])

        # persistent tensors
        qts = [const.tile([P, S], bf, tag=f"qt{eb}", name=f"qt{eb}") for eb in range(2)]
        kts = [const.tile([P, S], bf, tag=f"kt{eb}", name=f"kt{eb}") for eb in range(2)]
        # V augmented with a ones column per head: [s-part, sk-tile, 4*(64+1)]
        vaug = const.tile([P, NSK, HPC * (DK + 1)], bf, tag="vaug")
        vaug4 = vaug.rearrange("p t (h e) -> p t h e", h=HPC)
        nc.vector.memset(vaug4[:, :, :, DK], 1.0)
        # normalized attention values, laid out as Wo lhsT [e%128, e//128, s]
        vals = const.tile([P, 2, S], bf, tag="vals")

        def proj_qk_piece(w_sb, eb, sc, dst):
            """dst[:, chunk sc] = rope(W.T @ x.T) for e-block eb."""
            cols = slice(sc * SC, (sc + 1) * SC)
            ps = pa.tile([P, SC], f32, tag="pa")
            for ks in range(KS):
                nc.tensor.matmul(
                    ps[:],
                    lhsT=w_sb[:, ks, eb * P:(eb + 1) * P],
                    rhs=xss[sc][:, ks, :],
                    start=(ks == 0), stop=(ks == KS - 1),
                )
            t = work.tile([P, SC], bf, tag="t")
            nc.vector.tensor_mul(out=t[:], in0=ps[:], in1=cs_sb[:, cols])
            v = work.tile([P, SC], bf, tag="v")
            nc.vector.tensor_mul(out=v[:], in0=ps[:], in1=sn_sb[:, cols])
            u = work.tile([P, SC], bf, tag="u")
            nc.vector.stream_shuffle(out=u[:], in_=v[:], mask=SHUF16)
            nc.gpsimd.tensor_add(out=dst[:, cols], in0=t[:], in1=u[:])

        def proj_v_piece(sc, s4):
            """V for one 128-token block into its vaug slot."""
            sst = sc * 4 + s4
            ps = pa.tile([P, SC], f32, tag="pa")
            pv256 = ps[:, 0:E]
            for ks in range(KS):
                nc.tensor.matmul(
                    pv256,
                    lhsT=xss[sc][:, ks, s4 * P:(s4 + 1) * P],
                    rhs=wv_sb[:, ks, :],
                    start=(ks == 0), stop=(ks == KS - 1),
                )
            nc.vector.tensor_copy(
                out=vaug4[:, sst, :, 0:DK],
                in_=pv256.rearrange("p (h e) -> p h e", h=HPC),
            )

        def outproj_piece(j, s4, tail):
            """out[sq block, :] = vals.T @ woT for 128 tokens."""
            sq = j * 4 + s4
            for n2 in range(2):
                ps = pa.tile([P, SC], f32, tag="pa")
                for ks2 in range(2):
                    nc.tensor.matmul(
                        ps[:],
                        lhsT=vals[:, ks2, sq * P:(sq + 1) * P],
                        rhs=wo_sb[:, ks2, n2 * SC:(n2 + 1) * SC],
                        start=(ks2 == 0), stop=(ks2 == 1),
                    )
                ostg = work.tile([P, SC], bf, tag="ostg", name="ostg")
                if tail and n2 == 1:
                    nc.scalar.copy(out=ostg[:], in_=ps[:])
                else:
                    nc.vector.tensor_copy(out=ostg[:], in_=ps[:])
                nc.sync.dma_start(
                    out=out[sq * P:(sq + 1) * P, n2 * SC:(n2 + 1) * SC],
                    in_=ostg[:])

        def attention_col(j, hp, filler, fill_every):
            """Attention for sq column j, head pair hp (heads 2hp, 2hp+1).

            Emits score pair i together with P@V pair i-1 (one-group software
            pipeline); pulls a filler piece every `fill_every` groups.
            """
            jcols = slice(j * SC, (j + 1) * SC)
            ntiles = 4 * j + 4
            pvts = [pvp.tile([P, SC], f32, tag=f"pv{par}", name=f"pv{par}")
                    for par in range(2)]

            def emit_pv(prev):
                pexp, off, i = prev
                for par in range(2):
                    hl = 2 * hp + par
                    nc.tensor.matmul(
                        pvts[par][0:DK + 1, off:],
                        lhsT=vaug[:, i, hl * (DK + 1):(hl + 1) * (DK + 1)],
                        rhs=pexp[:, par, off:],
                        start=(i == 0), stop=(i == ntiles - 1),
                    )

            pend = []
            for i in range(ntiles):
                c = i - 4 * j
                off = c * P if c > 0 else 0
                stp = pb.tile([P, 2, SC], f32, tag="pb", name="stp")
                for par in range(2):
                    nc.tensor.matmul(
                        stp[:, par, off:],
                        lhsT=kts[hp][64 * par:64 * par + 64, i * P:(i + 1) * P],
                        rhs=qts[hp][64 * par:64 * par + 64,
                                    j * SC + off:(j + 1) * SC],
                        start=True, stop=True,
                    )
                if len(pend) >= 2:
                    emit_pv(pend.pop(0))
                pexp = pexp_pool.tile([P, 2, SC], bf, tag="pexp", name="pexp")
                nc.scalar.activation(out=pexp[:, :, off:], in_=stp[:, :, off:],
                                     func=Exp)
                if c >= 0:  # diagonal subtile: triangular mask
                    for par in range(2):
                        sl = pexp[:, par, c * P:(c + 1) * P]
                        nc.gpsimd.tensor_mul(out=sl, in0=sl, in1=tri_sb[:])
                pend.append((pexp, off, i))
                if filler and (i % fill_every == fill_every - 1):
                    filler.pop(0)()
            while pend:
                emit_pv(pend.pop(0))

            # normalize by the softmax denominator (row DK of pvts)
            for par in range(2):
                rsb = work.tile([P, SC], f32, tag="rsb")
                rb = work.tile([P, SC], f32, tag="rb")
                lsb = work.tile([P, SC], f32, tag="lsb", name="lsb")
                # cross-partition DVE copy: PSUM p64 -> SBUF p0; fast
                # reciprocal works from SBUF p0. Keeps ACT free for exp.
                nc.vector.tensor_copy(out=lsb[0:1, :], in_=pvts[par][DK:DK + 1, :])
                nc.vector.reciprocal_approx_fast(out=rsb[0:1, :],
                                                 in_=lsb[0:1, :])
                nc.gpsimd.partition_broadcast(rb[0:DK, :], rsb[0:1, :],
                                              channels=DK)
                dst = vals[64 * par:64 * par + 64, hp, jcols]
                if par == 0:
                    nc.vector.tensor_mul(out=dst, in0=pvts[par][0:DK, :],
                                         in1=rb[0:DK, :])
                else:
                    stg = work.tile([P, SC], bf, tag="stg")
                    nc.vector.tensor_mul(out=stg[0:DK, :], in0=pvts[par][0:DK, :],
                                         in1=rb[0:DK, :])
                    nc.sync.dma_start(out=dst, in_=stg[0:DK, :])

        # chunk 0 Q/K projections up front; V(0) becomes column-0 filler
        proj_qk_piece(wq_sb, 0, 0, qts[0])
        proj_qk_piece(wk_sb, 0, 0, kts[0])
        proj_qk_piece(wq_sb, 1, 0, qts[1])
        proj_qk_piece(wk_sb, 1, 0, kts[1])

        opt = []  # deferrable outproj pieces (vals persist; roll backwards)
        for sc in range(NSC):
            mand = []  # must complete during this sc iteration
            if sc == 0:
                for s4 in range(4):
                    mand.append(lambda s4=s4: proj_v_piece(0, s4))
            if sc + 1 < NSC:
                mand.append(lambda sc=sc: proj_qk_piece(wq_sb, 0, sc + 1, qts[0]))
                mand.append(lambda sc=sc: proj_qk_piece(wk_sb, 0, sc + 1, kts[0]))
                mand.append(lambda sc=sc: proj_qk_piece(wq_sb, 1, sc + 1, qts[1]))
                mand.append(lambda sc=sc: proj_qk_piece(wk_sb, 1, sc + 1, kts[1]))
                for s4 in range(4):
                    mand.append(lambda sc=sc, s4=s4: proj_v_piece(sc + 1, s4))
            ng = 4 * sc + 4
            pieces = mand + opt
            opt = []
            h = (len(pieces) + 1) // 2
            f0, f1 = pieces[:h], pieces[h:]
            fe0 = max(1, ng // (len(f0) + 1)) if f0 else ng
            fe1 = max(1, ng // (len(f1) + 1)) if f1 else ng
            attention_col(sc, 0, f0, fe0)
            if f1:
                f1.pop(0)()  # cover the hp0 -> hp1 normalize transition
            attention_col(sc, 1, f1, fe1)
            for f in f0 + f1:  # leftovers must land before next chunk's column
                f()
            if sc >= 1:
                for s4 in range(4):
                    opt.append(lambda sc=sc, s4=s4: outproj_piece(sc - 1, s4, False))
        for f in opt:
            f()
        for s4 in range(4):
            outproj_piece(NSC - 1, s4, True)

    nc.compile()
    return nc


def get_nc():
    if "nc" not in _CACHE:
        _CACHE["nc"] = _build_nc()
    return _CACHE["nc"]


def make_in_maps(x, Wq, Wk, Wv, Wo, token_positions, rope_theta):
    """Host-side sharding: per-core input dict (bf16, pre-transposed/permuted)."""
    x = np.asarray(x, np.float32)
    Wq = np.asarray(Wq, np.float32)
    Wk = np.asarray(Wk, np.float32)
    Wv = np.asarray(Wv, np.float32)
    Wo = np.asarray(Wo, np.float32)
    pos = np.asarray(token_positions).astype(np.float32)
    theta = float(np.asarray(rope_theta))

    # RoPE row layout per head (64 rows = 2 SBUF quadrants of 32):
    # quadrant q holds [evens of pairs 16q..16q+15, odds of same pairs], so
    # the rotate-half partner is 16 rows away inside the same quadrant.
    perm = []
    sign = np.empty(DK, np.float32)
    pairidx = np.empty(DK, np.int64)
    r = 0
    for q in range(2):
        for p in range(16 * q, 16 * q + 16):
            perm.append(2 * p)
            sign[r] = -1.0
            pairidx[r] = p
            r += 1
        for p in range(16 * q, 16 * q + 16):
            perm.append(2 * p + 1)
            sign[r] = 1.0
            pairidx[r] = p
            r += 1
    perm = np.array(perm)

    freqs = theta ** (-np.arange(DK // 2, dtype=np.float32) / (DK // 2))
    ang = pos[:, None] * freqs[None, :]          # [S, 32]
    cos_t = np.cos(ang).T.astype(np.float32)     # [32, S]
    sin_t = np.sin(ang).T.astype(np.float32)
    cs64 = cos_t[pairidx]                        # [64, S]
    sn64 = sin_t[pairidx] * sign[:, None]        # [64, S]
    # sn multiplies the pre-shuffle tensor: shuffle(q0*sn_pre) == swap(q0)*sn
    sig = np.arange(DK)
    sig = (sig // 32) * 32 + ((sig % 32) + 16) % 32
    snp64 = sn64[sig]
    cs_t = np.tile(cs64, (2, 1)).astype(BF)      # [128, S]
    sn_t = np.tile(snp64, (2, 1)).astype(BF)

    tri_t = np.tril(np.ones((P, P), np.float32)).T.astype(BF)  # keep p<=f

    in_maps = []
    for c in range(8):
        b, g = c // 4, c % 4
        hs = slice(g * E, (g + 1) * E)

        def prep_qk(W, scale):
            Wl = W[hs].reshape(HPC, DK, D)[:, perm, :].reshape(E, D) * scale
            return np.ascontiguousarray(Wl.T).astype(BF)

        in_maps.append({
            "xT": np.ascontiguousarray(x[b].T).astype(BF),
            "wq": prep_qk(Wq, 1.0 / np.sqrt(DK)),
            "wk": prep_qk(Wk, 1.0),
            "wv": np.ascontiguousarray(Wv[hs].T).astype(BF),
            "wo": np.ascontiguousarray(Wo[:, hs].T).astype(BF),
            "cs": cs_t, "sn": sn_t, "tri": tri_t,
        })
    return in_maps


def kernel(x, Wq, Wk, Wv, Wo, token_positions, rope_theta):
    nc = get_nc()
    in_maps = make_in_maps(x, Wq, Wk, Wv, Wo, token_positions, rope_theta)
    from concourse.bass_utils import run_bass_kernel_spmd
    r = run_bass_kernel_spmd(nc, in_maps, core_ids=list(range(8)))
    outs = [np.asarray(m["out"], np.float32) for m in r.results]
    full = np.stack([sum(outs[0:4]), sum(outs[4:8])], 0)
    return full.astype(np.float32)
